# revision 1
# baseline (speedup 1.0000x reference)
"""LightGCN-style GNN message passing on 8 Trainium2 NeuronCores.

Algorithm (matches the reference):
    deg  = bincount(dst);  dinv = rsqrt(max(deg, 1))
    x_{l+1} = dinv * (A @ (dinv * x_l))          (3 layers, A = binary adjacency)
    z_l = l2_normalize(x_l);  Z = concat(z_0..z_3);  Y = Z @ W.T + b
    return Y[senders], Y[receivers]

Key factorization: with xs_l = dinv * x_l, messages need no per-edge scale
(norm = dinv[src]*dinv[dst] splits into a pre-scale of the gathered table and
a post-scale of the scattered rows), and l2_normalize(xs_l) == l2_normalize(x_l)
since dinv > 0. So only the xs tables are ever materialized (bf16).

Sharding: destination-sharded. Core i owns N/8 dst rows; its edges are grouped
by (src-chunk, dst-block-of-128). Per 128-edge tile:
  - dma_gather 128 rows of xs (bf16, 256B rows) from the chunk window of the
    replicated table (int16 gather indices => 4 windows),
  - one-hot S[p, m] = (dst_local[p] == m) built on DVE via iota + is_equal,
  - PE matmul psum[block] += S.T @ msgs accumulates the segment sum on-chip.
Post-scale by dinv^2 on ACT -> bf16 own slice -> AllGather -> next layer.
Final: gather the 4 xs tables at the output rows this core handles,
l2-normalize, PE-transpose, matmul with W^T (bf16), add bias, DMA out.
The host only computes degrees and integer index/schedule arrays; all FLOAT
work on emb/W/b happens on device.
"""

import numpy as np
import ml_dtypes

import concourse.bacc as bacc
import concourse.mybir as mybir
import concourse.tile as tile

F32 = mybir.dt.float32
BF16 = mybir.dt.bfloat16
I16 = mybir.dt.int16
I32 = mybir.dt.int32

D = 128             # feature dim
NL = 3              # message passing layers
NC = 8              # cores
BLK = 128           # dst block (psum partition dim)
NCH = 4             # source chunks (int16 gather index limit)


class Cfg:
    def __init__(self, N, E, NOUT, GCALL=4096, SLAB=14):
        self.N = N
        self.E = E
        self.NOUT = NOUT
        self.GCALL = GCALL
        self.PER = N // NC
        self.NB = (self.PER + BLK - 1) // BLK
        self.SEG = self.NB * BLK
        self.NTOT = NC * self.SEG
        assert self.NTOT % NCH == 0
        self.CHUNK = self.NTOT // NCH
        assert self.CHUNK <= 32768, "int16 gather index overflow"
        assert self.CHUNK % 16 == 0
        self.OPC = NOUT // NC
        self.SLAB = min(SLAB, self.NB)


FULL = Cfg(N=100000, E=1600000, NOUT=16384)


def _ceil(a, b):
    return (a + b - 1) // b


def _wrap16(idx):
    """int16 [L] -> [128, L//16] wrapped in 16 partitions, replicated x8."""
    return np.tile(idx.reshape(-1, 16).T, (8, 1)).copy()


def _prep(cfg, emb, edge_index, senders, receivers, W, b):
    N, E, PER, SEG, NB, CHUNK = cfg.N, cfg.E, cfg.PER, cfg.SEG, cfg.NB, cfg.CHUNK
    src = np.asarray(edge_index[0], np.int64)
    dst = np.asarray(edge_index[1], np.int64)
    senders = np.asarray(senders, np.int64)
    receivers = np.asarray(receivers, np.int64)

    deg = np.bincount(dst, minlength=N).astype(np.float32)
    deg = np.maximum(deg, 1.0)
    dinv = (1.0 / np.sqrt(deg)).astype(np.float32)
    dinv2 = (dinv * dinv).astype(np.float32)

    # --- edge schedule -----------------------------------------------------
    core_of = dst // PER
    r = dst % PER
    blk = r // BLK
    dloc = r % BLK
    spn = (src // PER) * SEG + (src % PER)       # padded row of the source
    ch = spn // CHUNK
    sidx = (spn - ch * CHUNK).astype(np.int16)   # in-window gather index

    cell = (core_of * NCH + ch) * NB + blk       # [E] global (core,chunk,block)
    ncell = NC * NCH * NB
    counts = np.bincount(cell, minlength=ncell)
    T_cb = _ceil(counts.reshape(NC, NCH, NB), BLK).max(axis=0)  # [NCH, NB]

    tt_c = T_cb.sum(axis=1)                      # tiles per chunk stream
    ctile_off = np.concatenate([[0], np.cumsum(tt_c)[:-1]])
    TT = int(tt_c.sum())                         # total tiles per core
    TOTLEN = TT * BLK

    # tile base of each (c,b) within the concatenated per-core stream
    tb = np.zeros((NCH, NB), np.int64)
    for c in range(NCH):
        tb[c] = ctile_off[c] + np.concatenate([[0], np.cumsum(T_cb[c])[:-1]])
    base_cb = (tb * BLK).reshape(-1)             # [NCH*NB] edge-position base

    order = np.argsort(cell, kind="stable")
    cell_sorted = cell[order]
    starts = np.concatenate([[0], np.cumsum(counts)[:-1]])
    rank = np.arange(E, dtype=np.int64) - starts[cell_sorted]
    # cell % (NCH*NB) is the (chunk, block) flat index
    epos = base_cb[cell_sorted % (NCH * NB)] + rank

    eidx_arrs, edloc_arrs = [], []
    src_sorted = sidx[order]
    dloc_sorted = dloc[order]
    core_sorted = core_of[order]
    for i in range(NC):
        m = core_sorted == i
        ia = np.zeros(TOTLEN, np.int16)
        da = np.full(TOTLEN, -1.0, np.float32)
        ia[epos[m]] = src_sorted[m]
        da[epos[m]] = dloc_sorted[m]
        eidx_arrs.append(_wrap16(ia))                      # [128, TOTLEN//16]
        edloc_arrs.append(da.reshape(TT, BLK).T.copy())    # [128, TT]

    # --- output-row schedule ----------------------------------------------
    OPC = cfg.OPC
    ids = [np.concatenate([senders[OPC * i:OPC * (i + 1)],
                           receivers[OPC * i:OPC * (i + 1)]]) for i in range(NC)]
    fpn = [(x // PER) * SEG + (x % PER) for x in ids]
    fch = [x // CHUNK for x in fpn]
    gcnt = np.array([[int((fch[i] == c).sum()) for c in range(NCH)]
                     for i in range(NC)])
    OUT_T_c = _ceil(gcnt, BLK).max(axis=0)       # [NCH] tiles, shared
    OUT_T = int(OUT_T_c.sum())
    OUTLEN = OUT_T * BLK
    fbase = np.concatenate([[0], np.cumsum(OUT_T_c)[:-1]]) * BLK

    fidx_arrs, pos_arrs = [], []
    for i in range(NC):
        fo = np.argsort(fch[i], kind="stable")
        pos = np.zeros(2 * OPC, np.int64)
        ia = np.zeros(OUTLEN, np.int16)
        for c in range(NCH):
            m = fo[fch[i][fo] == c]
            p = fbase[c] + np.arange(len(m))
            pos[m] = p
            ia[p] = (fpn[i][m] - c * CHUNK).astype(np.int16)
        fidx_arrs.append(_wrap16(ia))            # [128, OUTLEN//16]
        pos_arrs.append(pos)

    # --- per-core dense inputs --------------------------------------------
    in_maps = []
    for i in range(NC):
        eo = np.zeros((SEG, D), np.float32)
        eo[:PER] = emb[PER * i:PER * (i + 1)]
        dv = np.zeros(SEG, np.float32)
        dv[:PER] = dinv[PER * i:PER * (i + 1)]
        dv2 = np.zeros(SEG, np.float32)
        dv2[:PER] = dinv2[PER * i:PER * (i + 1)]
        in_maps.append({
            "emb_own": eo,
            "dinv_col": dv.reshape(NB, BLK).T.copy(),
            "dinv2_col": dv2.reshape(NB, BLK).T.copy(),
            "eidx": eidx_arrs[i],
            "edloc": edloc_arrs[i],
            "fidx": fidx_arrs[i],
            "wt": np.ascontiguousarray(W.T).astype(ml_dtypes.bfloat16),
            "bb": np.broadcast_to(b, (128, 4 * D)).astype(np.float32).copy(),
        })

    meta = {
        "T_cb": T_cb, "ctile_off": ctile_off.astype(int),
        "len_c": [int(t) * BLK for t in tt_c],
        "TT": TT, "TOTLEN": TOTLEN, "OUT_T_c": OUT_T_c.astype(int),
        "OUT_T": OUT_T, "OUTLEN": OUTLEN,
    }
    return in_maps, meta, pos_arrs


def _build(cfg, meta, single=False, repeat=1):
    SEG, NB, NTOT, CHUNK, GCALL, SLAB = (cfg.SEG, cfg.NB, cfg.NTOT, cfg.CHUNK,
                                         cfg.GCALL, cfg.SLAB)
    T_cb = meta["T_cb"]
    ctile_off = meta["ctile_off"]
    len_c = meta["len_c"]
    TT = meta["TT"]
    TOTLEN = meta["TOTLEN"]
    OUT_T_c = meta["OUT_T_c"]
    OUT_T = meta["OUT_T"]
    OUTLEN = meta["OUTLEN"]
    off16 = np.concatenate([[0], np.cumsum([l // 16 for l in len_c])[:-1]]).astype(int)
    foff16 = np.concatenate(
        [[0], np.cumsum([int(t) * BLK // 16 for t in OUT_T_c])[:-1]]).astype(int)
    fbase_t = np.concatenate([[0], np.cumsum(OUT_T_c)[:-1]]).astype(int)

    nc = bacc.Bacc("TRN2", target_bir_lowering=False, debug=False,
                   enable_asserts=False, num_devices=(1 if single else NC))

    def allgather(l):
        if single:
            # timing-only stand-in: copy the own slice into the full table
            nc.sync.dma_start(xs_full[l][:SEG, :], xs_own[l][:])
        else:
            nc.gpsimd.collective_compute(
                "AllGather", mybir.AluOpType.bypass, replica_groups=RG,
                ins=[xs_own[l][:]], outs=[xs_full[l][:]])

    emb_own = nc.dram_tensor("emb_own", [SEG, D], F32, kind="ExternalInput")
    dinv_col = nc.dram_tensor("dinv_col", [128, NB], F32, kind="ExternalInput")
    dinv2_col = nc.dram_tensor("dinv2_col", [128, NB], F32, kind="ExternalInput")
    eidx = nc.dram_tensor("eidx", [128, TOTLEN // 16], I16, kind="ExternalInput")
    edloc = nc.dram_tensor("edloc", [128, TT], F32, kind="ExternalInput")
    fidx = nc.dram_tensor("fidx", [128, OUTLEN // 16], I16, kind="ExternalInput")
    wt = nc.dram_tensor("wt", [4 * D, 4 * D], BF16, kind="ExternalInput")
    bb = nc.dram_tensor("bb", [128, 4 * D], F32, kind="ExternalInput")
    y = nc.dram_tensor("y", [OUTLEN, 4 * D], F32, kind="ExternalOutput")

    xs_own = [nc.dram_tensor(f"xs_own{l}", [SEG, D], BF16) for l in range(NL + 1)]
    xs_full = [nc.dram_tensor(f"xs_full{l}", [NTOT, D], BF16, addr_space="Shared")
               for l in range(NL + 1)]
    RG = [list(range(NC))]

    with tile.TileContext(nc) as tc:
        with tc.tile_pool(name="const", bufs=1) as cpool:
            eidx_sb = cpool.tile([128, TOTLEN // 16], I16, tag="eidx")
            nc.sync.dma_start(eidx_sb[:], eidx[:])
            edloc_sb = cpool.tile([128, TT], F32, tag="edloc")
            nc.sync.dma_start(edloc_sb[:], edloc[:])
            fidx_sb = cpool.tile([128, OUTLEN // 16], I16, tag="fidx")
            nc.sync.dma_start(fidx_sb[:], fidx[:])
            dinv_sb = cpool.tile([128, NB], F32, tag="dinv")
            nc.sync.dma_start(dinv_sb[:], dinv_col[:])
            dinv2_sb = cpool.tile([128, NB], F32, tag="dinv2")
            nc.sync.dma_start(dinv2_sb[:], dinv2_col[:])
            wt_sb = cpool.tile([128, 4, 4 * D], BF16, tag="wt")
            nc.sync.dma_start(wt_sb[:], wt[:].rearrange("(l k) o -> k l o", k=128))
            bb_sb = cpool.tile([128, 4 * D], F32, tag="bb")
            nc.sync.dma_start(bb_sb[:], bb[:])

            iota_i = cpool.tile([128, 128], I32, tag="iota_i")
            nc.gpsimd.iota(iota_i[:], pattern=[[1, 128]], base=0,
                           channel_multiplier=0)
            iota_bf = cpool.tile([128, 128], BF16, tag="iota_bf")
            nc.vector.tensor_copy(iota_bf[:], iota_i[:])
            pidx_i = cpool.tile([128, 1], I32, tag="pidx_i")
            nc.gpsimd.iota(pidx_i[:], pattern=[[0, 1]], base=0,
                           channel_multiplier=1)
            pidx_f = cpool.tile([128, 1], F32, tag="pidx_f")
            nc.vector.tensor_copy(pidx_f[:], pidx_i[:])
            ident = cpool.tile([128, 128], BF16, tag="ident")
            nc.vector.tensor_scalar(ident[:], iota_bf[:], pidx_f[:], None,
                                    mybir.AluOpType.is_equal)

            for _rep in range(repeat):
              with (
                tc.tile_pool(name="gath", bufs=3) as gpool,
                tc.tile_pool(name="sone", bufs=4) as spool,
                tc.tile_pool(name="stag", bufs=2) as stpool,
                tc.tile_pool(name="eps", bufs=4, space="PSUM") as ppool,
              ):
                # ---- xs_0 = dinv * emb -------------------------------------
                embv = emb_own[:].rearrange("(s p) d -> p s d", p=128)
                xs0v = xs_own[0][:].rearrange("(s p) d -> p s d", p=128)
                for s0 in range(0, NB, SLAB):
                    n = min(SLAB, NB - s0)
                    slab = stpool.tile([128, SLAB, D], F32, tag="emb_slab")
                    nc.sync.dma_start(slab[:, :n, :], embv[:, s0:s0 + n, :])
                    stg = stpool.tile([128, SLAB, D], BF16, tag="stg0")
                    for j in range(n):
                        nc.vector.tensor_scalar(
                            stg[:, j, :], slab[:, j, :],
                            dinv_sb[:, s0 + j:s0 + j + 1], None,
                            mybir.AluOpType.mult)
                    nc.sync.dma_start(xs0v[:, s0:s0 + n, :], stg[:, :n, :])
                allgather(0)

                # ---- message-passing layers -------------------------------
                for l in range(NL):
                    windows = [xs_full[l][c * CHUNK:(c + 1) * CHUNK, :]
                               for c in range(NCH)]
                    xsov = xs_own[l + 1][:].rearrange("(s p) d -> p s d", p=128)
                    gtiles = {}
                    next_call = [0] * NCH

                    def ensure(c, tile_hi, gtiles=gtiles, next_call=next_call,
                               windows=windows):
                        while next_call[c] * (GCALL // BLK) < tile_hi:
                            k = next_call[c]
                            n_idx = min(GCALL, len_c[c] - k * GCALL)
                            gt = gpool.tile([128, GCALL // BLK, D], BF16,
                                            tag=f"g{c}")
                            nc.gpsimd.dma_gather(
                                gt[:, :n_idx // BLK, :], windows[c],
                                eidx_sb[:, off16[c] + k * (GCALL // 16):
                                        off16[c] + k * (GCALL // 16) + n_idx // 16],
                                num_idxs=n_idx, num_idxs_reg=n_idx,
                                elem_size=D, single_packet=(n_idx <= 1024))
                            gtiles[(c, k)] = gt
                            next_call[c] += 1

                    pos = [0] * NCH
                    stg = None
                    for b in range(NB):
                        if b % SLAB == 0:
                            stg = stpool.tile([128, SLAB, D], BF16, tag="stgL")
                        nmm = int(sum(T_cb[c][b] for c in range(NCH)))
                        ps = ppool.tile([128, D], F32, tag="ps")
                        mm = 0
                        for c in range(NCH):
                            ensure(c, pos[c] + int(T_cb[c][b]))
                            for t in range(int(T_cb[c][b])):
                                tg = pos[c] + t
                                k, s = divmod(tg, GCALL // BLK)
                                S = spool.tile([128, 128], BF16, tag="S")
                                col = int(ctile_off[c]) + tg
                                nc.vector.tensor_scalar(
                                    S[:], iota_bf[:],
                                    edloc_sb[:, col:col + 1], None,
                                    mybir.AluOpType.is_equal)
                                nc.tensor.matmul(
                                    ps[:], lhsT=S[:],
                                    rhs=gtiles[(c, k)][:, s, :],
                                    start=(mm == 0), stop=(mm == nmm - 1))
                                mm += 1
                            pos[c] += int(T_cb[c][b])
                        nc.scalar.mul(stg[:, b % SLAB, :], ps[:],
                                      dinv2_sb[:, b:b + 1])
                        if b % SLAB == SLAB - 1 or b == NB - 1:
                            s0 = b - b % SLAB
                            nc.sync.dma_start(xsov[:, s0:b + 1, :],
                                              stg[:, :b - s0 + 1, :])
                    allgather(l + 1)

              # ---- final: gather + normalize + concat + MLP + select --------
              with (
                tc.tile_pool(name="fg", bufs=1) as fpool,
                tc.tile_pool(name="fz", bufs=3) as zpool,
                tc.tile_pool(name="fpt", bufs=2, space="PSUM") as ptpool,
                tc.tile_pool(name="fpy", bufs=2, space="PSUM") as pypool,
              ):
                fg = fpool.tile([128, NL + 1, OUT_T, D], BF16, tag="fg")
                for l in range(NL + 1):
                    for c in range(NCH):
                        n_idx = int(OUT_T_c[c]) * BLK
                        nc.gpsimd.dma_gather(
                            fg[:, l, fbase_t[c]:fbase_t[c] + int(OUT_T_c[c]), :],
                            xs_full[l][c * CHUNK:(c + 1) * CHUNK, :],
                            fidx_sb[:, foff16[c]:foff16[c] + n_idx // 16],
                            num_idxs=n_idx, num_idxs_reg=n_idx, elem_size=D,
                            single_packet=(n_idx <= 1024))

                for ot in range(OUT_T):
                    scr = zpool.tile([128, 128], F32, tag="scr")
                    ssq = zpool.tile([128, NL + 1], F32, tag="ssq")
                    for l in range(NL + 1):
                        nc.scalar.activation(
                            scr[:], fg[:, l, ot, :],
                            mybir.ActivationFunctionType.Square,
                            accum_out=ssq[:, l:l + 1])
                    nrm = zpool.tile([128, NL + 1], F32, tag="nrm")
                    nc.scalar.sqrt(nrm[:], ssq[:])
                    nc.vector.tensor_scalar_max(nrm[:], nrm[:], 1e-12)
                    rinv = zpool.tile([128, NL + 1], F32, tag="rinv")
                    nc.vector.reciprocal(rinv[:], nrm[:])
                    zT = zpool.tile([128, NL + 1, 128], BF16, tag="zT")
                    for l in range(NL + 1):
                        z = zpool.tile([128, 128], BF16, tag="z")
                        nc.vector.tensor_scalar(
                            z[:], fg[:, l, ot, :], rinv[:, l:l + 1], None,
                            mybir.AluOpType.mult)
                        pt = ptpool.tile([128, 128], BF16, tag="pt")
                        nc.tensor.transpose(pt[:], z[:], ident[:])
                        nc.scalar.copy(zT[:, l, :], pt[:])
                    py = pypool.tile([128, 4 * D], F32, tag="py")
                    for l in range(NL + 1):
                        nc.tensor.matmul(py[:], lhsT=zT[:, l, :],
                                         rhs=wt_sb[:, l, :],
                                         start=(l == 0), stop=(l == NL))
                    ysb = zpool.tile([128, 4 * D], F32, tag="ysb")
                    nc.vector.tensor_add(ysb[:], py[:], bb_sb[:])
                    nc.sync.dma_start(y[ot * 128:(ot + 1) * 128, :], ysb[:])

    nc.compile()
    return nc


def _run(inputs, trace=False, cfg=FULL):
    from concourse.bass_utils import run_bass_kernel_spmd

    emb = np.asarray(inputs["emb"], np.float32)
    edge_index = np.asarray(inputs["edge_index"])
    senders = np.asarray(inputs["senders"])
    receivers = np.asarray(inputs["receivers"])
    W = np.asarray(inputs["W"], np.float32)
    b = np.asarray(inputs["b"], np.float32)

    in_maps, meta, pos_arrs = _prep(cfg, emb, edge_index, senders, receivers, W, b)
    nc = _build(cfg, meta)
    res = run_bass_kernel_spmd(nc, in_maps, list(range(NC)), trace=trace)

    s_out = np.empty((cfg.NOUT, 4 * D), np.float32)
    r_out = np.empty((cfg.NOUT, 4 * D), np.float32)
    OPC = cfg.OPC
    for i in range(NC):
        yv = res.results[i]["y"]
        s_out[OPC * i:OPC * (i + 1)] = yv[pos_arrs[i][:OPC]]
        r_out[OPC * i:OPC * (i + 1)] = yv[pos_arrs[i][OPC:]]
    return (s_out, r_out), res


def kernel(**inputs):
    out, _ = _run(inputs, trace=False)
    return out



# revision 8
# speedup vs baseline: 19.7514x; 19.7514x over previous
"""LightGCN-style GNN message passing on 8 Trainium2 NeuronCores.

Algorithm (matches the reference):
    deg  = bincount(dst);  dinv = rsqrt(max(deg, 1))
    x_{l+1} = dinv * (A @ (dinv * x_l))          (3 layers, A = binary adjacency)
    z_l = l2_normalize(x_l);  Z = concat(z_0..z_3);  Y = Z @ W.T + b
    return Y[senders], Y[receivers]

Key factorization: with xs_l = dinv * x_l, messages need no per-edge scale
(norm = dinv[src]*dinv[dst] splits into a pre-scale of the gathered table and
a post-scale of the scattered rows), and l2_normalize(xs_l) == l2_normalize(x_l)
since dinv > 0. So only the xs tables are ever materialized (bf16).

Sharding: destination-sharded. Core i owns N/8 dst rows. The node table is
split into NCH row-windows; each window is AllGathered separately so the
collective overlaps the producing layer's tail. Per 128-edge tile:
  - dma_gather 128 rows of xs (bf16, 256B rows) from the window table
    (int16 gather indices),
  - one-hot S[p, m] = (dst_local[p] == m) built on DVE via iota + is_equal,
  - PE matmul psum[block] += S.T @ msgs accumulates the segment sum on-chip.
Post-scale by dinv^2 on ACT -> bf16 own slice -> windowed AllGather.
Layers 1 and 2 cover all N dst rows. Layer 3 is computed ONLY for the
~4096 output rows this core's senders/receivers need (destination slots),
so xs_3 needs no AllGather and stays SBUF-resident.
Final: gather xs_0..xs_2 at the output rows, take xs_3 from SBUF,
l2-normalize, PE-transpose, matmul with W^T (bf16), add bias, DMA out.
The host only computes degrees and integer index/schedule arrays; all FLOAT
work on emb/W/b happens on device.
"""

import numpy as np
import ml_dtypes

import concourse.bacc as bacc
import concourse.mybir as mybir
import concourse.tile as tile

F32 = mybir.dt.float32
BF16 = mybir.dt.bfloat16
I16 = mybir.dt.int16
I32 = mybir.dt.int32

D = 128             # feature dim
NL = 3              # message passing layers
NC = 8              # cores
BLK = 128           # dst block (psum partition dim)
NCH = 4             # table row-windows (also int16 gather index limit)


class Cfg:
    def __init__(self, N, E, NOUT, GCALL=4096):
        self.N = N
        self.E = E
        self.NOUT = NOUT
        self.GCALL = GCALL
        self.PER = N // NC
        self.NB = (self.PER + BLK - 1) // BLK
        self.SEG = self.NB * BLK
        # row-windows: split NB blocks into NCH nearly-equal spans
        q, rem = divmod(self.NB, NCH)
        self.WB = [q + (1 if w < rem else 0) for w in range(NCH)]
        self.B0 = np.concatenate([[0], np.cumsum(self.WB)[:-1]]).astype(int)
        self.L = [wb * BLK for wb in self.WB]
        for lw in self.L:
            assert NC * lw <= 32768, "int16 gather index overflow"
        self.OPC = NOUT // NC


FULL = Cfg(N=100000, E=1600000, NOUT=16384)


def _ceil(a, b):
    return (a + b - 1) // b


def _wrap16(idx):
    """int16 [L] -> [128, L//16] wrapped in 16 partitions, replicated x8."""
    return np.tile(idx.reshape(-1, 16).T, (8, 1)).copy()


def _streams(cfg, wsrc, sidx, core, blk, dloc, nblk):
    """Shared-schedule per-core per-window edge streams.

    Returns (T_cb [NCH, nblk] shared tile counts, per-core eidx arrays
    (one per window), per-core edloc [BLK, TT] arrays (cols grouped
    window-major by (w, block, tile))).
    """
    cell = (core * NCH + wsrc) * nblk + blk
    ncell = NC * NCH * nblk
    counts = np.bincount(cell, minlength=ncell)
    T_cb = _ceil(counts.reshape(NC, NCH, nblk), BLK).max(axis=0)  # [NCH, nblk]
    T_cb = np.maximum(T_cb, 1)

    tbase = np.zeros((NCH, nblk), np.int64)      # tile base within window
    for w in range(NCH):
        tbase[w] = np.concatenate([[0], np.cumsum(T_cb[w])[:-1]])
    TT_w = T_cb.sum(axis=1)                      # tiles per window
    wcoloff = np.concatenate([[0], np.cumsum(TT_w)[:-1]]).astype(int)
    TT = int(TT_w.sum())

    order = np.argsort(cell, kind="stable")
    cell_sorted = cell[order]
    starts = np.concatenate([[0], np.cumsum(counts)[:-1]])
    rank = np.arange(len(cell), dtype=np.int64) - starts[cell_sorted]
    wb_flat = cell_sorted % (NCH * nblk)         # (w, b) flat
    w_of = wb_flat // nblk
    b_of = wb_flat % nblk
    pos_in_w = tbase[w_of, b_of] * BLK + rank    # position within window stream

    sidx_sorted = sidx[order]
    dloc_sorted = dloc[order]
    core_sorted = core[order]

    eidx_arrs, edloc_arrs = [], []
    for i in range(NC):
        m = core_sorted == i
        ias = []
        for w in range(NCH):
            mw = m & (w_of == w)
            ia = np.zeros(int(TT_w[w]) * BLK, np.int16)
            ia[pos_in_w[mw]] = sidx_sorted[mw]
            ias.append(_wrap16(ia))
        da = np.full(TT * BLK, -1.0, np.float32)
        gcol = (wcoloff[w_of] + tbase[w_of, b_of]) * BLK + rank
        da[gcol[m]] = dloc_sorted[m]
        eidx_arrs.append(ias)
        edloc_arrs.append(da.reshape(TT, BLK).T.copy())   # [128, TT]
    meta = {
        "T_cb": T_cb, "tbase": tbase, "TT_w": TT_w.astype(int),
        "wcoloff": wcoloff, "TT": TT,
        "LENW": [int(t) * BLK for t in TT_w],
    }
    return meta, eidx_arrs, edloc_arrs


def _prep(cfg, emb, edge_index, senders, receivers, W, b):
    N, E, PER, SEG, NB = cfg.N, cfg.E, cfg.PER, cfg.SEG, cfg.NB
    src = np.asarray(edge_index[0], np.int64)
    dst = np.asarray(edge_index[1], np.int64)
    senders = np.asarray(senders, np.int64)
    receivers = np.asarray(receivers, np.int64)

    deg = np.bincount(dst, minlength=N).astype(np.float32)
    deg = np.maximum(deg, 1.0)
    dinv = (1.0 / np.sqrt(deg)).astype(np.float32)
    dinv2 = (dinv * dinv).astype(np.float32)

    WBc = np.cumsum(cfg.WB)
    Larr = np.array(cfg.L, np.int64)
    B0arr = np.asarray(cfg.B0, np.int64) * BLK

    def src_map(s):
        j = s // PER
        r = s % PER
        w = np.searchsorted(WBc, r // BLK, side="right")
        return w, (j * Larr[w] + (r - B0arr[w])).astype(np.int16)

    wsrc, sidx = src_map(src)
    core = dst // PER
    rd = dst % PER
    dblk = rd // BLK
    dloc = rd % BLK

    meta12, eidx12, edloc12 = _streams(cfg, wsrc, sidx, core, dblk, dloc, NB)

    # --- output slots ------------------------------------------------------
    OPC = cfg.OPC
    ids = [np.concatenate([senders[OPC * i:OPC * (i + 1)],
                           receivers[OPC * i:OPC * (i + 1)]]) for i in range(NC)]
    fw = [np.searchsorted(WBc, (x % PER) // BLK, side="right") for x in ids]
    fsx = [((x // PER) * Larr[w] + (x % PER - B0arr[w])).astype(np.int16)
           for x, w in zip(ids, fw)]
    gcnt = np.array([[int((fw[i] == w).sum()) for w in range(NCH)]
                     for i in range(NC)])
    OT_w = _ceil(gcnt, BLK).max(axis=0)          # [NCH], shared
    OUT_T = int(OT_w.sum())
    OUTLEN = OUT_T * BLK
    fbase = np.concatenate([[0], np.cumsum(OT_w)[:-1]]).astype(int)

    fidx_arrs, pos_arrs, dv2o_arrs = [], [], []
    out_core, out_slot = [], []                  # for layer-3 contributions
    for i in range(NC):
        fo = np.argsort(fw[i], kind="stable")
        pos = np.zeros(2 * OPC, np.int64)
        ia = np.zeros(OUTLEN, np.int16)
        for w in range(NCH):
            mm = fo[fw[i][fo] == w]
            p = fbase[w] * BLK + np.arange(len(mm))
            pos[mm] = p
            ia[p] = fsx[i][mm]
        fidx_arrs.append(_wrap16(ia))
        pos_arrs.append(pos)
        dv = np.zeros(OUTLEN, np.float32)
        dv[pos] = dinv2[ids[i]]
        dv2o_arrs.append(dv.reshape(OUT_T, BLK).T.copy())   # [128, OUT_T]
        out_core.append(np.full(2 * OPC, i, np.int64))
        out_slot.append(pos)

    # --- layer-3 contributions: edges into each core's output rows --------
    c_wsrc, c_sidx, c_core, c_blk, c_dloc = [], [], [], [], []
    for i in range(NC):
        sr_order = np.argsort(ids[i], kind="stable")
        sr = ids[i][sr_order]
        le = np.searchsorted(sr, dst, side="left")
        ri = np.searchsorted(sr, dst, side="right")
        cnt = ri - le
        tot = int(cnt.sum())
        eexp = np.repeat(np.arange(E), cnt)
        base = np.repeat(le, cnt)
        cum = np.concatenate([[0], np.cumsum(cnt)[:-1]])
        off = np.arange(tot, dtype=np.int64) - np.repeat(cum, cnt)
        s_orig = sr_order[base + off]
        spos = pos_arrs[i][s_orig]
        c_wsrc.append(wsrc[eexp])
        c_sidx.append(sidx[eexp])
        c_core.append(np.full(tot, i, np.int64))
        c_blk.append(spos // BLK)
        c_dloc.append(spos % BLK)
    meta3, eidx3, edloc3 = _streams(
        cfg, np.concatenate(c_wsrc), np.concatenate(c_sidx),
        np.concatenate(c_core), np.concatenate(c_blk),
        np.concatenate(c_dloc), OUT_T)

    # --- per-core dense inputs --------------------------------------------
    in_maps = []
    for i in range(NC):
        eo = np.zeros((SEG, D), np.float32)
        eo[:PER] = emb[PER * i:PER * (i + 1)]
        dv = np.zeros(SEG, np.float32)
        dv[:PER] = dinv[PER * i:PER * (i + 1)]
        dv2 = np.zeros(SEG, np.float32)
        dv2[:PER] = dinv2[PER * i:PER * (i + 1)]
        mp = {
            "emb_own": eo,
            "dinv_col": dv.reshape(NB, BLK).T.copy(),
            "dinv2_col": dv2.reshape(NB, BLK).T.copy(),
            "dinv2o_col": dv2o_arrs[i],
            "edloc": edloc12[i],
            "edloc3": edloc3[i],
            "fidx": fidx_arrs[i],
            "wt": np.ascontiguousarray(W.T).astype(ml_dtypes.bfloat16),
            "bb": np.broadcast_to(b, (128, 4 * D)).astype(np.float32).copy(),
        }
        for w in range(NCH):
            mp[f"eidx{w}"] = eidx12[i][w]
            mp[f"eidx3_{w}"] = eidx3[i][w]
        in_maps.append(mp)

    meta = {
        "m12": meta12, "m3": meta3,
        "OT_w": OT_w.astype(int), "OUT_T": OUT_T, "OUTLEN": OUTLEN,
        "fbase": fbase,
        "TT": meta12["TT"], "TT3": meta3["TT"],
    }
    return in_maps, meta, pos_arrs


def _build(cfg, meta, single=False, repeat=1, nocoll=False):
    SEG, NB, GCALL = cfg.SEG, cfg.NB, cfg.GCALL
    WB, B0, L = cfg.WB, cfg.B0, cfg.L
    m12, m3 = meta["m12"], meta["m3"]
    OT_w, OUT_T, OUTLEN = meta["OT_w"], meta["OUT_T"], meta["OUTLEN"]
    fbase = meta["fbase"]
    foff16 = np.concatenate(
        [[0], np.cumsum([int(t) * BLK // 16 for t in OT_w])[:-1]]).astype(int)

    nc = bacc.Bacc("TRN2", target_bir_lowering=False, debug=False,
                   enable_asserts=False, num_devices=(1 if single else NC),
                   num_swdge_queues=4)

    emb_own = nc.dram_tensor("emb_own", [SEG, D], F32, kind="ExternalInput")
    dinv_col = nc.dram_tensor("dinv_col", [128, NB], F32, kind="ExternalInput")
    dinv2_col = nc.dram_tensor("dinv2_col", [128, NB], F32, kind="ExternalInput")
    dinv2o_col = nc.dram_tensor("dinv2o_col", [128, OUT_T], F32,
                                kind="ExternalInput")
    eidx_in = [nc.dram_tensor(f"eidx{w}", [128, m12["LENW"][w] // 16], I16,
                              kind="ExternalInput") for w in range(NCH)]
    eidx3_in = [nc.dram_tensor(f"eidx3_{w}", [128, m3["LENW"][w] // 16], I16,
                               kind="ExternalInput") for w in range(NCH)]
    edloc = nc.dram_tensor("edloc", [128, m12["TT"]], F32, kind="ExternalInput")
    edloc3 = nc.dram_tensor("edloc3", [128, m3["TT"]], F32, kind="ExternalInput")
    fidx = nc.dram_tensor("fidx", [128, OUTLEN // 16], I16, kind="ExternalInput")
    wt = nc.dram_tensor("wt", [4 * D, 4 * D], BF16, kind="ExternalInput")
    bb = nc.dram_tensor("bb", [128, 4 * D], F32, kind="ExternalInput")
    y = nc.dram_tensor("y", [OUTLEN, 4 * D], F32, kind="ExternalOutput")

    xs_own = [[nc.dram_tensor(f"xs_own{l}_{w}", [L[w], D], BF16)
               for w in range(NCH)] for l in range(NL)]
    xs_win = [[nc.dram_tensor(f"xs_win{l}_{w}", [NC * L[w], D], BF16,
                              addr_space="Shared")
               for w in range(NCH)] for l in range(NL)]
    RG = [list(range(NC))]

    def allgather(l, w):
        if single or nocoll:
            nc.sync.dma_start(xs_win[l][w][:L[w], :], xs_own[l][w][:])
        else:
            nc.gpsimd.collective_compute(
                "AllGather", mybir.AluOpType.bypass, replica_groups=RG,
                ins=[xs_own[l][w][:]], outs=[xs_win[l][w][:]])

    with tile.TileContext(nc) as tc:
        with tc.tile_pool(name="const", bufs=1) as cpool:
            eidx_sb = []
            for w in range(NCH):
                t = cpool.tile([128, m12["LENW"][w] // 16], I16, tag=f"eidx{w}")
                nc.sync.dma_start(t[:], eidx_in[w][:])
                eidx_sb.append(t)
            eidx3_sb = []
            for w in range(NCH):
                t = cpool.tile([128, m3["LENW"][w] // 16], I16, tag=f"eidx3{w}")
                nc.sync.dma_start(t[:], eidx3_in[w][:])
                eidx3_sb.append(t)
            edloc_sb = cpool.tile([128, m12["TT"]], F32, tag="edloc")
            nc.sync.dma_start(edloc_sb[:], edloc[:])
            edloc3_sb = cpool.tile([128, m3["TT"]], F32, tag="edloc3")
            nc.sync.dma_start(edloc3_sb[:], edloc3[:])
            fidx_sb = cpool.tile([128, OUTLEN // 16], I16, tag="fidx")
            nc.sync.dma_start(fidx_sb[:], fidx[:])
            dinv_sb = cpool.tile([128, NB], F32, tag="dinv")
            nc.sync.dma_start(dinv_sb[:], dinv_col[:])
            dinv2_sb = cpool.tile([128, NB], F32, tag="dinv2")
            nc.sync.dma_start(dinv2_sb[:], dinv2_col[:])
            dinv2o_sb = cpool.tile([128, OUT_T], F32, tag="dinv2o")
            nc.sync.dma_start(dinv2o_sb[:], dinv2o_col[:])
            wt_sb = cpool.tile([128, 4, 4 * D], BF16, tag="wt")
            nc.sync.dma_start(wt_sb[:], wt[:].rearrange("(l k) o -> k l o", k=128))
            bb_sb = cpool.tile([128, 4 * D], F32, tag="bb")
            nc.sync.dma_start(bb_sb[:], bb[:])

            iota_i = cpool.tile([128, 128], I32, tag="iota_i")
            nc.gpsimd.iota(iota_i[:], pattern=[[1, 128]], base=0,
                           channel_multiplier=0)
            iota_bf = cpool.tile([128, 128], BF16, tag="iota_bf")
            nc.vector.tensor_copy(iota_bf[:], iota_i[:])
            pidx_i = cpool.tile([128, 1], I32, tag="pidx_i")
            nc.gpsimd.iota(pidx_i[:], pattern=[[0, 1]], base=0,
                           channel_multiplier=1)
            pidx_f = cpool.tile([128, 1], F32, tag="pidx_f")
            nc.vector.tensor_copy(pidx_f[:], pidx_i[:])
            ident = cpool.tile([128, 128], BF16, tag="ident")
            nc.vector.tensor_scalar(ident[:], iota_bf[:], pidx_f[:], None,
                                    mybir.AluOpType.is_equal)
            xs3_sb = cpool.tile([128, OUT_T, D], BF16, tag="xs3")

            for _rep in range(repeat):
              with (
                tc.tile_pool(name="gath", bufs=3) as gpool,
                tc.tile_pool(name="sone", bufs=4) as spool,
                tc.tile_pool(name="stag", bufs=2) as stpool,
                tc.tile_pool(name="eps", bufs=4, space="PSUM") as ppool,
              ):
                # ---- xs_0 = dinv * emb, windowed ---------------------------
                embv = emb_own[:].rearrange("(s p) d -> p s d", p=128)
                for w in range(NCH):
                    xs0v = xs_own[0][w][:].rearrange("(s p) d -> p s d", p=128)
                    for s0 in range(0, WB[w], 13):
                        n = min(13, WB[w] - s0)
                        g0 = int(B0[w]) + s0
                        slab = stpool.tile([128, 13, D], F32, tag="emb_slab")
                        nc.sync.dma_start(slab[:, :n, :], embv[:, g0:g0 + n, :])
                        stg = stpool.tile([128, 13, D], BF16, tag="stg0")
                        for jj in range(n):
                            nc.vector.tensor_scalar(
                                stg[:, jj, :], slab[:, jj, :],
                                dinv_sb[:, g0 + jj:g0 + jj + 1], None,
                                mybir.AluOpType.mult)
                        nc.sync.dma_start(xs0v[:, s0:s0 + n, :], stg[:, :n, :])
                    allgather(0, w)

                # ---- message-passing layers 1, 2 ---------------------------
                def run_layer(src_tabs, mm, eidx_tiles, nblk, sink):
                    """Stream gathers per window; per dst block accumulate in
                    psum over all windows; sink(b, ps) consumes the result."""
                    T_cb, tbase, wcoloff = mm["T_cb"], mm["tbase"], mm["wcoloff"]
                    LENW = mm["LENW"]
                    off16 = [0] * NCH
                    gtiles = {}
                    next_call = [0] * NCH

                    def ensure(w, tile_hi):
                        while next_call[w] * (GCALL // BLK) < tile_hi:
                            k = next_call[w]
                            n_idx = min(GCALL, LENW[w] - k * GCALL)
                            gt = gpool.tile([128, GCALL // BLK, D], BF16,
                                            tag=f"g{w}")
                            nc.gpsimd.dma_gather(
                                gt[:, :n_idx // BLK, :], src_tabs[w],
                                eidx_tiles[w][:, k * (GCALL // 16):
                                              k * (GCALL // 16) + n_idx // 16],
                                num_idxs=n_idx, num_idxs_reg=n_idx,
                                elem_size=D, single_packet=(n_idx <= 1024),
                                queue_num=w)
                            gtiles[(w, k)] = gt
                            next_call[w] += 1

                    for b in range(nblk):
                        nmm = int(T_cb[:, b].sum())
                        ps = ppool.tile([128, D], F32, tag="ps")
                        mmi = 0
                        for w in range(NCH):
                            ensure(w, int(tbase[w][b]) + int(T_cb[w][b]))
                            for t in range(int(T_cb[w][b])):
                                tg = int(tbase[w][b]) + t
                                k, s = divmod(tg, GCALL // BLK)
                                S = spool.tile([128, 128], BF16, tag="S")
                                col = int(wcoloff[w]) + tg
                                nc.vector.tensor_scalar(
                                    S[:], iota_bf[:],
                                    edloc_sb_cur[:, col:col + 1], None,
                                    mybir.AluOpType.is_equal)
                                nc.tensor.matmul(
                                    ps[:], lhsT=S[:],
                                    rhs=gtiles[(w, k)][:, s, :],
                                    start=(mmi == 0), stop=(mmi == nmm - 1))
                                mmi += 1
                        sink(b, ps)

                for l in range(NL - 1):
                    edloc_sb_cur = edloc_sb
                    stg_ref = [None]
                    wcur = [0]

                    def sink12(b, ps, l=l, stg_ref=stg_ref, wcur=wcur):
                        w = wcur[0]
                        if b == int(B0[w]):
                            stg_ref[0] = stpool.tile([128, max(WB), D], BF16,
                                                     name="stgL", tag="stgL")
                        nc.scalar.mul(stg_ref[0][:, b - int(B0[w]), :], ps[:],
                                      dinv2_sb[:, b:b + 1])
                        if b - int(B0[w]) == WB[w] - 1:
                            xsov = xs_own[l + 1][w][:].rearrange(
                                "(s p) d -> p s d", p=128)
                            nc.sync.dma_start(xsov[:, :WB[w], :],
                                              stg_ref[0][:, :WB[w], :])
                            allgather(l + 1, w)
                            wcur[0] += 1

                    run_layer([xs_win[l][w][:] for w in range(NCH)], m12,
                              eidx_sb, NB, sink12)

                # ---- layer 3: restricted to output slots -------------------
                edloc_sb_cur = edloc3_sb

                def sink3(b, ps):
                    nc.scalar.mul(xs3_sb[:, b, :], ps[:],
                                  dinv2o_sb[:, b:b + 1])

                run_layer([xs_win[NL - 1][w][:] for w in range(NCH)], m3,
                          eidx3_sb, OUT_T, sink3)

              # ---- final: gather + normalize + concat + MLP ----------------
              with (
                tc.tile_pool(name="fg", bufs=1) as fpool,
                tc.tile_pool(name="fz", bufs=3) as zpool,
                tc.tile_pool(name="fpt", bufs=2, space="PSUM") as ptpool,
                tc.tile_pool(name="fpy", bufs=2, space="PSUM") as pypool,
              ):
                fg = fpool.tile([128, NL, OUT_T, D], BF16, tag="fg")
                for l in range(NL):
                    for w in range(NCH):
                        n_idx = int(OT_w[w]) * BLK
                        nc.gpsimd.dma_gather(
                            fg[:, l, fbase[w]:fbase[w] + int(OT_w[w]), :],
                            xs_win[l][w][:],
                            fidx_sb[:, foff16[w]:foff16[w] + n_idx // 16],
                            num_idxs=n_idx, num_idxs_reg=n_idx, elem_size=D,
                            single_packet=(n_idx <= 1024), queue_num=w)

                for ot in range(OUT_T):
                    def lay(l):
                        return xs3_sb[:, ot, :] if l == NL else fg[:, l, ot, :]
                    scr = zpool.tile([128, 128], F32, tag="scr")
                    ssq = zpool.tile([128, NL + 1], F32, tag="ssq")
                    for l in range(NL + 1):
                        nc.scalar.activation(
                            scr[:], lay(l),
                            mybir.ActivationFunctionType.Square,
                            accum_out=ssq[:, l:l + 1])
                    nrm = zpool.tile([128, NL + 1], F32, tag="nrm")
                    nc.scalar.sqrt(nrm[:], ssq[:])
                    nc.vector.tensor_scalar_max(nrm[:], nrm[:], 1e-12)
                    rinv = zpool.tile([128, NL + 1], F32, tag="rinv")
                    nc.vector.reciprocal(rinv[:], nrm[:])
                    zT = zpool.tile([128, NL + 1, 128], BF16, tag="zT")
                    for l in range(NL + 1):
                        z = zpool.tile([128, 128], BF16, tag="z")
                        nc.vector.tensor_scalar(
                            z[:], lay(l), rinv[:, l:l + 1], None,
                            mybir.AluOpType.mult)
                        pt = ptpool.tile([128, 128], BF16, tag="pt")
                        nc.tensor.transpose(pt[:], z[:], ident[:])
                        nc.scalar.copy(zT[:, l, :], pt[:])
                    py = pypool.tile([128, 4 * D], F32, tag="py")
                    for l in range(NL + 1):
                        nc.tensor.matmul(py[:], lhsT=zT[:, l, :],
                                         rhs=wt_sb[:, l, :],
                                         start=(l == 0), stop=(l == NL))
                    ysb = zpool.tile([128, 4 * D], F32, tag="ysb")
                    nc.vector.tensor_add(ysb[:], py[:], bb_sb[:])
                    nc.sync.dma_start(y[ot * 128:(ot + 1) * 128, :], ysb[:])

    nc.compile()
    return nc


def _run(inputs, trace=False, cfg=FULL):
    from concourse.bass_utils import run_bass_kernel_spmd

    emb = np.asarray(inputs["emb"], np.float32)
    edge_index = np.asarray(inputs["edge_index"])
    senders = np.asarray(inputs["senders"])
    receivers = np.asarray(inputs["receivers"])
    W = np.asarray(inputs["W"], np.float32)
    b = np.asarray(inputs["b"], np.float32)

    in_maps, meta, pos_arrs = _prep(cfg, emb, edge_index, senders, receivers, W, b)
    nc = _build(cfg, meta)
    res = run_bass_kernel_spmd(nc, in_maps, list(range(NC)), trace=trace)

    s_out = np.empty((cfg.NOUT, 4 * D), np.float32)
    r_out = np.empty((cfg.NOUT, 4 * D), np.float32)
    OPC = cfg.OPC
    for i in range(NC):
        yv = res.results[i]["y"]
        s_out[OPC * i:OPC * (i + 1)] = yv[pos_arrs[i][:OPC]]
        r_out[OPC * i:OPC * (i + 1)] = yv[pos_arrs[i][OPC:]]
    return (s_out, r_out), res


def kernel(**inputs):
    out, _ = _run(inputs, trace=False)
    return out


# revision 13
# speedup vs baseline: 38.0210x; 1.9250x over previous
"""LightGCN-style GNN message passing on 8 Trainium2 NeuronCores.

Algorithm (matches the reference):
    deg  = bincount(dst);  dinv = rsqrt(max(deg, 1))
    x_{l+1} = dinv * (A @ (dinv * x_l))          (3 layers, A = binary adjacency)
    z_l = l2_normalize(x_l);  Z = concat(z_0..z_3);  Y = Z @ W.T + b
    return Y[senders], Y[receivers]

Key factorization: with xs_l = dinv * x_l, messages need no per-edge scale
(norm = dinv[src]*dinv[dst] splits into a pre-scale of the gathered table and
a post-scale of the scattered rows), and l2_normalize(xs_l) == l2_normalize(x_l)
since dinv > 0. So only the xs tables are ever materialized (bf16).

Sharding: destination-sharded. Core i owns N/8 dst rows. The node table is
split into NCH row-windows; each window is AllGathered separately so the
collective overlaps the producing layer's tail. Per 128-edge tile:
  - dma_gather 128 rows of xs (bf16, 256B rows) from the window table
    (int16 gather indices),
  - one-hot S[p, m] = (dst_local[p] == m) built on DVE via iota + is_equal,
  - PE matmul psum[block] += S.T @ msgs accumulates the segment sum on-chip.
Post-scale by dinv^2 on ACT -> bf16 own slice -> windowed AllGather.
Layers 1 and 2 cover all N dst rows. Layer 3 is computed ONLY for the
~4096 output rows this core's senders/receivers need (destination slots),
so xs_3 needs no AllGather and stays SBUF-resident.
Final: gather xs_0..xs_2 at the output rows, take xs_3 from SBUF,
l2-normalize, PE-transpose, matmul with W^T (bf16), add bias, DMA out.
The host only computes degrees and integer index/schedule arrays; all FLOAT
work on emb/W/b happens on device.
"""

import numpy as np
import ml_dtypes

import concourse.bacc as bacc
import concourse.mybir as mybir
import concourse.tile as tile

F32 = mybir.dt.float32
BF16 = mybir.dt.bfloat16
I16 = mybir.dt.int16
I32 = mybir.dt.int32

D = 128             # feature dim
NL = 3              # message passing layers
NC = 8              # cores
BLK = 128           # dst block (psum partition dim)
NCH = 4             # table row-windows (also int16 gather index limit)


class Cfg:
    def __init__(self, N, E, NOUT, GCALL=4096):
        self.N = N
        self.E = E
        self.NOUT = NOUT
        self.GCALL = GCALL
        self.PER = N // NC
        self.NB = (self.PER + BLK - 1) // BLK
        self.SEG = self.NB * BLK
        # row-windows: split NB blocks into NCH nearly-equal spans
        q, rem = divmod(self.NB, NCH)
        self.WB = [q + (1 if w < rem else 0) for w in range(NCH)]
        self.B0 = np.concatenate([[0], np.cumsum(self.WB)[:-1]]).astype(int)
        self.L = [wb * BLK for wb in self.WB]
        for lw in self.L:
            assert NC * lw <= 32768, "int16 gather index overflow"
        self.OPC = NOUT // NC


FULL = Cfg(N=100000, E=1600000, NOUT=16384)


def _ceil(a, b):
    return (a + b - 1) // b


def _wrap16(idx):
    """int16 [L] -> [128, L//16] wrapped in 16 partitions, replicated x8."""
    return np.tile(idx.reshape(-1, 16).T, (8, 1)).copy()


def _streams(cfg, wsrc, sidx, core, blk, dloc, nblk):
    """Shared-schedule per-core per-window edge streams.

    Returns (T_cb [NCH, nblk] shared tile counts, per-core eidx arrays
    (one per window), per-core edloc [BLK, TT] arrays (cols grouped
    window-major by (w, block, tile))).
    """
    cell = (core * NCH + wsrc) * nblk + blk
    ncell = NC * NCH * nblk
    counts = np.bincount(cell, minlength=ncell)
    T_cb = _ceil(counts.reshape(NC, NCH, nblk), BLK).max(axis=0)  # [NCH, nblk]
    T_cb = np.maximum(T_cb, 1)

    tbase = np.zeros((NCH, nblk), np.int64)      # tile base within window
    for w in range(NCH):
        tbase[w] = np.concatenate([[0], np.cumsum(T_cb[w])[:-1]])
    TT_w = T_cb.sum(axis=1)                      # tiles per window
    wcoloff = np.concatenate([[0], np.cumsum(TT_w)[:-1]]).astype(int)
    TT = int(TT_w.sum())

    order = np.argsort(cell, kind="stable")
    cell_sorted = cell[order]
    starts = np.concatenate([[0], np.cumsum(counts)[:-1]])
    rank = np.arange(len(cell), dtype=np.int64) - starts[cell_sorted]
    wb_flat = cell_sorted % (NCH * nblk)         # (w, b) flat
    w_of = wb_flat // nblk
    b_of = wb_flat % nblk
    pos_in_w = tbase[w_of, b_of] * BLK + rank    # position within window stream

    sidx_sorted = sidx[order]
    dloc_sorted = dloc[order]
    core_sorted = core[order]

    eidx_arrs, edloc_arrs = [], []
    for i in range(NC):
        m = core_sorted == i
        ias = []
        for w in range(NCH):
            mw = m & (w_of == w)
            ia = np.zeros(int(TT_w[w]) * BLK, np.int16)
            ia[pos_in_w[mw]] = sidx_sorted[mw]
            ias.append(_wrap16(ia))
        da = np.full(TT * BLK, -1.0, np.float32)
        gcol = (wcoloff[w_of] + tbase[w_of, b_of]) * BLK + rank
        da[gcol[m]] = dloc_sorted[m]
        eidx_arrs.append(ias)
        edloc_arrs.append(da.reshape(TT, BLK).T.copy())   # [128, TT]
    meta = {
        "T_cb": T_cb, "tbase": tbase, "TT_w": TT_w.astype(int),
        "wcoloff": wcoloff, "TT": TT,
        "LENW": [int(t) * BLK for t in TT_w],
    }
    return meta, eidx_arrs, edloc_arrs


def _prep(cfg, emb, edge_index, senders, receivers, W, b):
    N, E, PER, SEG, NB = cfg.N, cfg.E, cfg.PER, cfg.SEG, cfg.NB
    src = np.asarray(edge_index[0], np.int64)
    dst = np.asarray(edge_index[1], np.int64)
    senders = np.asarray(senders, np.int64)
    receivers = np.asarray(receivers, np.int64)

    deg = np.bincount(dst, minlength=N).astype(np.float32)
    deg = np.maximum(deg, 1.0)
    dinv = (1.0 / np.sqrt(deg)).astype(np.float32)
    dinv2 = (dinv * dinv).astype(np.float32)

    WBc = np.cumsum(cfg.WB)
    Larr = np.array(cfg.L, np.int64)
    B0arr = np.asarray(cfg.B0, np.int64) * BLK

    def src_map(s):
        j = s // PER
        r = s % PER
        w = np.searchsorted(WBc, r // BLK, side="right")
        return w, (j * Larr[w] + (r - B0arr[w])).astype(np.int16)

    wsrc, sidx = src_map(src)
    core = dst // PER
    rd = dst % PER
    dblk = rd // BLK
    dloc = rd % BLK

    meta12, eidx12, edloc12 = _streams(cfg, wsrc, sidx, core, dblk, dloc, NB)

    # --- output slots ------------------------------------------------------
    OPC = cfg.OPC
    ids = [np.concatenate([senders[OPC * i:OPC * (i + 1)],
                           receivers[OPC * i:OPC * (i + 1)]]) for i in range(NC)]
    fw = [np.searchsorted(WBc, (x % PER) // BLK, side="right") for x in ids]
    fsx = [((x // PER) * Larr[w] + (x % PER - B0arr[w])).astype(np.int16)
           for x, w in zip(ids, fw)]
    gcnt = np.array([[int((fw[i] == w).sum()) for w in range(NCH)]
                     for i in range(NC)])
    OT_w = _ceil(gcnt, BLK).max(axis=0)          # [NCH], shared
    OUT_T = int(OT_w.sum())
    OUTLEN = OUT_T * BLK
    fbase = np.concatenate([[0], np.cumsum(OT_w)[:-1]]).astype(int)

    fidx_arrs, pos_arrs, dv2o_arrs = [], [], []
    out_core, out_slot = [], []                  # for layer-3 contributions
    for i in range(NC):
        fo = np.argsort(fw[i], kind="stable")
        pos = np.zeros(2 * OPC, np.int64)
        ia = np.zeros(OUTLEN, np.int16)
        for w in range(NCH):
            mm = fo[fw[i][fo] == w]
            p = fbase[w] * BLK + np.arange(len(mm))
            pos[mm] = p
            ia[p] = fsx[i][mm]
        fidx_arrs.append(_wrap16(ia))
        pos_arrs.append(pos)
        dv = np.zeros(OUTLEN, np.float32)
        dv[pos] = dinv2[ids[i]]
        dv2o_arrs.append(dv.reshape(OUT_T, BLK).T.copy())   # [128, OUT_T]
        out_core.append(np.full(2 * OPC, i, np.int64))
        out_slot.append(pos)

    # --- layer-3 contributions: edges into each core's output rows --------
    c_wsrc, c_sidx, c_core, c_blk, c_dloc = [], [], [], [], []
    for i in range(NC):
        sr_order = np.argsort(ids[i], kind="stable")
        sr = ids[i][sr_order]
        le = np.searchsorted(sr, dst, side="left")
        ri = np.searchsorted(sr, dst, side="right")
        cnt = ri - le
        tot = int(cnt.sum())
        eexp = np.repeat(np.arange(E), cnt)
        base = np.repeat(le, cnt)
        cum = np.concatenate([[0], np.cumsum(cnt)[:-1]])
        off = np.arange(tot, dtype=np.int64) - np.repeat(cum, cnt)
        s_orig = sr_order[base + off]
        spos = pos_arrs[i][s_orig]
        c_wsrc.append(wsrc[eexp])
        c_sidx.append(sidx[eexp])
        c_core.append(np.full(tot, i, np.int64))
        c_blk.append(spos // BLK)
        c_dloc.append(spos % BLK)
    meta3, eidx3, edloc3 = _streams(
        cfg, np.concatenate(c_wsrc), np.concatenate(c_sidx),
        np.concatenate(c_core), np.concatenate(c_blk),
        np.concatenate(c_dloc), OUT_T)

    # --- per-core dense inputs --------------------------------------------
    in_maps = []
    for i in range(NC):
        eo = np.zeros((SEG, D), np.float32)
        eo[:PER] = emb[PER * i:PER * (i + 1)]
        dv = np.zeros(SEG, np.float32)
        dv[:PER] = dinv[PER * i:PER * (i + 1)]
        dv2 = np.zeros(SEG, np.float32)
        dv2[:PER] = dinv2[PER * i:PER * (i + 1)]
        mp = {
            "emb_own": eo,
            "dinv_col": dv.reshape(NB, BLK).T.copy(),
            "dinv2_col": dv2.reshape(NB, BLK).T.copy(),
            "dinv2o_col": dv2o_arrs[i],
            "edloc": edloc12[i],
            "edloc3": edloc3[i],
            "fidx": fidx_arrs[i],
            "wt": np.ascontiguousarray(W.T).astype(ml_dtypes.bfloat16),
            "bb": np.broadcast_to(b, (128, 4 * D)).astype(np.float32).copy(),
        }
        for w in range(NCH):
            mp[f"eidx{w}"] = eidx12[i][w]
            mp[f"eidx3_{w}"] = eidx3[i][w]
        in_maps.append(mp)

    meta = {
        "m12": meta12, "m3": meta3,
        "OT_w": OT_w.astype(int), "OUT_T": OUT_T, "OUTLEN": OUTLEN,
        "fbase": fbase,
        "TT": meta12["TT"], "TT3": meta3["TT"],
    }
    return in_maps, meta, pos_arrs


def _build(cfg, meta, single=False, repeat=1, nocoll=False, ablate=None):
    # ablate (timing experiments only, breaks correctness):
    #   'noS'      - one S-build per block instead of one per tile
    #   'nogather' - only the first gather call per window is issued
    #   'nomm'     - nogather + one matmul per block
    noS = ablate in ("noS", "noall")
    nogather = ablate in ("nogather", "nomm", "noall")
    nomm = ablate in ("nomm", "noall")
    SEG, NB, GCALL = cfg.SEG, cfg.NB, cfg.GCALL
    WB, B0, L = cfg.WB, cfg.B0, cfg.L
    m12, m3 = meta["m12"], meta["m3"]
    OT_w, OUT_T, OUTLEN = meta["OT_w"], meta["OUT_T"], meta["OUTLEN"]
    fbase = meta["fbase"]
    foff16 = np.concatenate(
        [[0], np.cumsum([int(t) * BLK // 16 for t in OT_w])[:-1]]).astype(int)

    nc = bacc.Bacc("TRN2", target_bir_lowering=False, debug=False,
                   enable_asserts=False, num_devices=(1 if single else NC),
                   num_swdge_queues=4)

    emb_own = nc.dram_tensor("emb_own", [SEG, D], F32, kind="ExternalInput")
    dinv_col = nc.dram_tensor("dinv_col", [128, NB], F32, kind="ExternalInput")
    dinv2_col = nc.dram_tensor("dinv2_col", [128, NB], F32, kind="ExternalInput")
    dinv2o_col = nc.dram_tensor("dinv2o_col", [128, OUT_T], F32,
                                kind="ExternalInput")
    eidx_in = [nc.dram_tensor(f"eidx{w}", [128, m12["LENW"][w] // 16], I16,
                              kind="ExternalInput") for w in range(NCH)]
    eidx3_in = [nc.dram_tensor(f"eidx3_{w}", [128, m3["LENW"][w] // 16], I16,
                               kind="ExternalInput") for w in range(NCH)]
    edloc = nc.dram_tensor("edloc", [128, m12["TT"]], F32, kind="ExternalInput")
    edloc3 = nc.dram_tensor("edloc3", [128, m3["TT"]], F32, kind="ExternalInput")
    fidx = nc.dram_tensor("fidx", [128, OUTLEN // 16], I16, kind="ExternalInput")
    wt = nc.dram_tensor("wt", [4 * D, 4 * D], BF16, kind="ExternalInput")
    bb = nc.dram_tensor("bb", [128, 4 * D], F32, kind="ExternalInput")
    y = nc.dram_tensor("y", [OUTLEN, 4 * D], F32, kind="ExternalOutput")

    xs_own = [[nc.dram_tensor(f"xs_own{l}_{w}", [L[w], D], BF16)
               for w in range(NCH)] for l in range(NL)]
    xs_win = [[nc.dram_tensor(f"xs_win{l}_{w}", [NC * L[w], D], BF16,
                              addr_space="Shared")
               for w in range(NCH)] for l in range(NL)]
    RG = [list(range(NC))]

    def allgather(l, w):
        if single or nocoll:
            nc.sync.dma_start(xs_win[l][w][:L[w], :], xs_own[l][w][:])
        else:
            nc.gpsimd.collective_compute(
                "AllGather", mybir.AluOpType.bypass, replica_groups=RG,
                ins=[xs_own[l][w][:]], outs=[xs_win[l][w][:]])

    with tile.TileContext(nc) as tc:
        with tc.tile_pool(name="const", bufs=1) as cpool:
            eidx_sb = []
            for w in range(NCH):
                t = cpool.tile([128, m12["LENW"][w] // 16], I16, tag=f"eidx{w}")
                nc.sync.dma_start(t[:], eidx_in[w][:])
                eidx_sb.append(t)
            eidx3_sb = []
            for w in range(NCH):
                t = cpool.tile([128, m3["LENW"][w] // 16], I16, tag=f"eidx3{w}")
                nc.sync.dma_start(t[:], eidx3_in[w][:])
                eidx3_sb.append(t)
            edloc_sb = cpool.tile([128, m12["TT"]], F32, tag="edloc")
            nc.sync.dma_start(edloc_sb[:], edloc[:])
            edloc3_sb = cpool.tile([128, m3["TT"]], F32, tag="edloc3")
            nc.sync.dma_start(edloc3_sb[:], edloc3[:])
            fidx_sb = cpool.tile([128, OUTLEN // 16], I16, tag="fidx")
            nc.sync.dma_start(fidx_sb[:], fidx[:])
            dinv_sb = cpool.tile([128, NB], F32, tag="dinv")
            nc.sync.dma_start(dinv_sb[:], dinv_col[:])
            dinv2_sb = cpool.tile([128, NB], F32, tag="dinv2")
            nc.sync.dma_start(dinv2_sb[:], dinv2_col[:])
            dinv2o_sb = cpool.tile([128, OUT_T], F32, tag="dinv2o")
            nc.sync.dma_start(dinv2o_sb[:], dinv2o_col[:])
            wt_sb = cpool.tile([128, 4, 4 * D], BF16, tag="wt")
            nc.sync.dma_start(wt_sb[:], wt[:].rearrange("(l k) o -> k l o", k=128))
            bb_sb = cpool.tile([128, 4 * D], F32, tag="bb")
            nc.sync.dma_start(bb_sb[:], bb[:])

            iota_i = cpool.tile([128, 128], I32, tag="iota_i")
            nc.gpsimd.iota(iota_i[:], pattern=[[1, 128]], base=0,
                           channel_multiplier=0)
            iota_bf = cpool.tile([128, 128], BF16, tag="iota_bf")
            nc.vector.tensor_copy(iota_bf[:], iota_i[:])
            pidx_i = cpool.tile([128, 1], I32, tag="pidx_i")
            nc.gpsimd.iota(pidx_i[:], pattern=[[0, 1]], base=0,
                           channel_multiplier=1)
            pidx_f = cpool.tile([128, 1], F32, tag="pidx_f")
            nc.vector.tensor_copy(pidx_f[:], pidx_i[:])
            ident = cpool.tile([128, 128], BF16, tag="ident")
            nc.vector.tensor_scalar(ident[:], iota_bf[:], pidx_f[:], None,
                                    mybir.AluOpType.is_equal)
            xs3_sb = cpool.tile([128, OUT_T, D], BF16, tag="xs3")

            TBMAX = int(max(m12["T_cb"].max(), m3["T_cb"].max()))
            iota_wide = cpool.tile([128, TBMAX, 128], BF16, tag="iota_wide")
            nc.vector.tensor_copy(
                iota_wide[:],
                iota_bf[:].unsqueeze(1).broadcast_to([128, TBMAX, 128]))

            for _rep in range(repeat):
              with (
                tc.tile_pool(name="gath", bufs=3) as gpool,
                tc.tile_pool(name="sone", bufs=4) as spool,
                tc.tile_pool(name="stag", bufs=2) as stpool,
                tc.tile_pool(name="eps", bufs=4, space="PSUM") as ppool,
              ):
                # ---- xs_0 = dinv * emb, windowed ---------------------------
                embv = emb_own[:].rearrange("(s p) d -> p s d", p=128)
                for w in range(NCH):
                    xs0v = xs_own[0][w][:].rearrange("(s p) d -> p s d", p=128)
                    for s0 in range(0, WB[w], 13):
                        n = min(13, WB[w] - s0)
                        g0 = int(B0[w]) + s0
                        slab = stpool.tile([128, 13, D], F32, tag="emb_slab")
                        nc.sync.dma_start(slab[:, :n, :], embv[:, g0:g0 + n, :])
                        stg = stpool.tile([128, 13, D], BF16, tag="stg0")
                        for jj in range(n):
                            nc.vector.tensor_scalar(
                                stg[:, jj, :], slab[:, jj, :],
                                dinv_sb[:, g0 + jj:g0 + jj + 1], None,
                                mybir.AluOpType.mult)
                        nc.sync.dma_start(xs0v[:, s0:s0 + n, :], stg[:, :n, :])
                    allgather(0, w)

                # ---- message-passing layers 1, 2 ---------------------------
                def run_layer(src_tabs, mm, eidx_tiles, nblk, sink):
                    """Stream gathers per window; per dst block accumulate in
                    psum over all windows; sink(b, ps) consumes the result."""
                    T_cb, tbase, wcoloff = mm["T_cb"], mm["tbase"], mm["wcoloff"]
                    LENW = mm["LENW"]
                    off16 = [0] * NCH
                    gtiles = {}
                    next_call = [0] * NCH

                    def ensure(w, tile_hi):
                        while next_call[w] * (GCALL // BLK) < tile_hi:
                            k = next_call[w]
                            if nogather and k > 0:
                                next_call[w] += 1
                                continue
                            n_idx = min(GCALL, LENW[w] - k * GCALL)
                            gt = gpool.tile([128, GCALL // BLK, D], BF16,
                                            tag=f"g{w}")
                            nc.gpsimd.dma_gather(
                                gt[:, :n_idx // BLK, :], src_tabs[w],
                                eidx_tiles[w][:, k * (GCALL // 16):
                                              k * (GCALL // 16) + n_idx // 16],
                                num_idxs=n_idx, num_idxs_reg=n_idx,
                                elem_size=D, single_packet=(n_idx <= 1024),
                                queue_num=w)
                            gtiles[(w, k)] = gt
                            next_call[w] += 1

                    for b in range(nblk):
                        nmm = 1 if nomm else int(T_cb[:, b].sum())
                        ps = ppool.tile([128, D], F32, tag="ps")
                        mmi = 0
                        for w in range(NCH):
                            T = int(T_cb[w][b])
                            ensure(w, int(tbase[w][b]) + T)
                            c0 = int(wcoloff[w]) + int(tbase[w][b])
                            Sw = spool.tile([128, TBMAX, 128], BF16,
                                            name="Sw", tag="Sw")
                            if not noS:
                                nc.vector.tensor_tensor(
                                    Sw[:, :T, :], iota_wide[:, :T, :],
                                    edloc_sb_cur[:, c0:c0 + T].unsqueeze(
                                        -1).broadcast_to([128, T, 128]),
                                    mybir.AluOpType.is_equal)
                            for t in range(T):
                                tg = int(tbase[w][b]) + t
                                k, s = divmod(tg, GCALL // BLK)
                                if nogather:
                                    k, s = 0, tg % (GCALL // BLK)
                                if mmi < nmm:
                                    nc.tensor.matmul(
                                        ps[:], lhsT=Sw[:, t, :],
                                        rhs=gtiles[(w, k)][:, s, :],
                                        start=(mmi == 0), stop=(mmi == nmm - 1))
                                mmi += 1
                        sink(b, ps)

                for l in range(NL - 1):
                    edloc_sb_cur = edloc_sb
                    stg_ref = [None]
                    wcur = [0]

                    def sink12(b, ps, l=l, stg_ref=stg_ref, wcur=wcur):
                        w = wcur[0]
                        if b == int(B0[w]):
                            stg_ref[0] = stpool.tile([128, max(WB), D], BF16,
                                                     name="stgL", tag="stgL")
                        nc.scalar.mul(stg_ref[0][:, b - int(B0[w]), :], ps[:],
                                      dinv2_sb[:, b:b + 1])
                        if b - int(B0[w]) == WB[w] - 1:
                            xsov = xs_own[l + 1][w][:].rearrange(
                                "(s p) d -> p s d", p=128)
                            nc.sync.dma_start(xsov[:, :WB[w], :],
                                              stg_ref[0][:, :WB[w], :])
                            allgather(l + 1, w)
                            wcur[0] += 1

                    run_layer([xs_win[l][w][:] for w in range(NCH)], m12,
                              eidx_sb, NB, sink12)

                # ---- layer 3: restricted to output slots -------------------
                edloc_sb_cur = edloc3_sb

                def sink3(b, ps):
                    nc.scalar.mul(xs3_sb[:, b, :], ps[:],
                                  dinv2o_sb[:, b:b + 1])

                run_layer([xs_win[NL - 1][w][:] for w in range(NCH)], m3,
                          eidx3_sb, OUT_T, sink3)

              # ---- final: gather + normalize + concat + MLP ----------------
              with (
                tc.tile_pool(name="fg", bufs=1) as fpool,
                tc.tile_pool(name="fz", bufs=3) as zpool,
                tc.tile_pool(name="fpt", bufs=2, space="PSUM") as ptpool,
                tc.tile_pool(name="fpy", bufs=2, space="PSUM") as pypool,
              ):
                fg = fpool.tile([128, NL, OUT_T, D], BF16, tag="fg")
                for l in range(NL):
                    for w in range(NCH):
                        n_idx = int(OT_w[w]) * BLK
                        nc.gpsimd.dma_gather(
                            fg[:, l, fbase[w]:fbase[w] + int(OT_w[w]), :],
                            xs_win[l][w][:],
                            fidx_sb[:, foff16[w]:foff16[w] + n_idx // 16],
                            num_idxs=n_idx, num_idxs_reg=n_idx, elem_size=D,
                            single_packet=(n_idx <= 1024), queue_num=w)

                for ot in range(OUT_T):
                    def lay(l):
                        return xs3_sb[:, ot, :] if l == NL else fg[:, l, ot, :]
                    scr = zpool.tile([128, 128], F32, tag="scr")
                    ssq = zpool.tile([128, NL + 1], F32, tag="ssq")
                    for l in range(NL + 1):
                        nc.scalar.activation(
                            scr[:], lay(l),
                            mybir.ActivationFunctionType.Square,
                            accum_out=ssq[:, l:l + 1])
                    nrm = zpool.tile([128, NL + 1], F32, tag="nrm")
                    nc.scalar.sqrt(nrm[:], ssq[:])
                    nc.vector.tensor_scalar_max(nrm[:], nrm[:], 1e-12)
                    rinv = zpool.tile([128, NL + 1], F32, tag="rinv")
                    nc.vector.reciprocal(rinv[:], nrm[:])
                    zT = zpool.tile([128, NL + 1, 128], BF16, tag="zT")
                    for l in range(NL + 1):
                        z = zpool.tile([128, 128], BF16, tag="z")
                        nc.vector.tensor_scalar(
                            z[:], lay(l), rinv[:, l:l + 1], None,
                            mybir.AluOpType.mult)
                        pt = ptpool.tile([128, 128], BF16, tag="pt")
                        nc.tensor.transpose(pt[:], z[:], ident[:])
                        nc.scalar.copy(zT[:, l, :], pt[:])
                    py = pypool.tile([128, 4 * D], F32, tag="py")
                    for l in range(NL + 1):
                        nc.tensor.matmul(py[:], lhsT=zT[:, l, :],
                                         rhs=wt_sb[:, l, :],
                                         start=(l == 0), stop=(l == NL))
                    ysb = zpool.tile([128, 4 * D], F32, tag="ysb")
                    nc.vector.tensor_add(ysb[:], py[:], bb_sb[:])
                    nc.sync.dma_start(y[ot * 128:(ot + 1) * 128, :], ysb[:])

    nc.compile()
    return nc


def _run(inputs, trace=False, cfg=FULL):
    from concourse.bass_utils import run_bass_kernel_spmd

    emb = np.asarray(inputs["emb"], np.float32)
    edge_index = np.asarray(inputs["edge_index"])
    senders = np.asarray(inputs["senders"])
    receivers = np.asarray(inputs["receivers"])
    W = np.asarray(inputs["W"], np.float32)
    b = np.asarray(inputs["b"], np.float32)

    in_maps, meta, pos_arrs = _prep(cfg, emb, edge_index, senders, receivers, W, b)
    nc = _build(cfg, meta)
    res = run_bass_kernel_spmd(nc, in_maps, list(range(NC)), trace=trace)

    s_out = np.empty((cfg.NOUT, 4 * D), np.float32)
    r_out = np.empty((cfg.NOUT, 4 * D), np.float32)
    OPC = cfg.OPC
    for i in range(NC):
        yv = res.results[i]["y"]
        s_out[OPC * i:OPC * (i + 1)] = yv[pos_arrs[i][:OPC]]
        r_out[OPC * i:OPC * (i + 1)] = yv[pos_arrs[i][OPC:]]
    return (s_out, r_out), res


def kernel(**inputs):
    out, _ = _run(inputs, trace=False)
    return out
